# revision 75
# baseline (speedup 1.0000x reference)
"""NetVLAD Trainium2 kernel (Bass/Tile), data-parallel over batch on 8 cores.

Problem (hardcoded): x [64, 128, 60, 60] f32, conv_w [64, 128] f32,
centroids [64, 128] f32 -> out [64, 8192] f32.

Per core: 8 samples, software-pipelined (phase1 of sample n+1 is emitted
before phase2 of sample n so no engine's program order stalls the next
sample). S = 3600 spatial positions padded to SP = 4096 (8 x 512 ss
chunks); phase 2 only touches the 29 real 128-position tiles.

Per sample (layouts chosen so every reduction is a free-dim reduction and
ScalarE stays on the natural_log_exp table set -- no ACT table reloads):
  phase1: ss[s] = sum_c x^2 via indicator-matmuls folded into one PSUM
          bank [chunk, 512]; 4 PE transposes + permuting copy -> t-order;
          inv = exp(-0.5 ln ss), norm = exp(0.5 ln ss), pads masked by
          affine_select (ss:=1, norm:=0)
  phase2: mm1 lhsT=x_tile[bf16] -> logits [s,64]; PE transposes -> X_T
          (bf16 PSUM, 2x-mode drains); lsc = logits*inv (DVE bcast);
          e = exp(lsc); sume = sum_k e; a' = e*(inv/sume) (GPSIMD);
          mm2 lhsT=a', rhs=[X_T | norm] -> [64,129] = [vlad1^T | asum];
          v = vlad1 - asum*cent; out = v * (0.125/sqrt(sum v^2)) via
          exp/ln (global L2 norm of 64 unit rows == 8 exactly)
"""

import os
import numpy as np

N, C, H, W, K = 64, 128, 60, 60, 64
S = H * W            # 3600
NS = 8               # samples per core
NCORES = 8
TP = 128             # s-tile size
T = 29               # real (non-pad) tiles per sample
SP = 4096            # padded positions: 8 x 512 ss-chunks, 32 tiles
PAD = SP - S         # 496

_cache = {}


def build_nc(variant="full", reps=1):
    # variant: full | dma_only | phase1 | no_mm2 | scalar_cast
    # reps>1 repeats the whole pipeline (for on-device timing via slope)
    import concourse.bass as bass
    import concourse.bacc as bacc
    import concourse.tile as tile
    from concourse import mybir
    from concourse.masks import make_identity

    f32 = mybir.dt.float32
    bf16 = mybir.dt.bfloat16
    AF = mybir.ActivationFunctionType
    ALU = mybir.AluOpType
    AX = mybir.AxisListType

    nc = bacc.Bacc("TRN2")
    x_in = nc.declare_dram_parameter("x", [NS, C, S], f32, isOutput=False)
    cw_in = nc.declare_dram_parameter("conv_w", [K, C], f32, isOutput=False)
    cent_in = nc.declare_dram_parameter("centroids", [K, C], f32, isOutput=False)
    out_ext = nc.declare_dram_parameter("out", [NS, K * C], f32, isOutput=True)

    from contextlib import ExitStack

    with tile.TileContext(nc) as tc, ExitStack() as ctx:
        singles = ctx.enter_context(tc.tile_pool(name="singles", bufs=1))

        if variant == "dma_only":
            acc = singles.tile([C, SP], f32)
            for n in range(NS):
                xt = singles.tile([C, SP], f32, tag=f"x{n}")
                nc.sync.dma_start(out=xt[:, :S], in_=x_in[n])
            nc.vector.tensor_copy(acc[:], xt[:])
            ov0 = singles.tile([K, C], f32)
            nc.vector.tensor_copy(ov0[:], acc[:K, :C])
            for n in range(NS):
                nc.sync.dma_start(
                    out=out_ext[n].rearrange("(k c) -> k c", c=C), in_=ov0[:]
                )
            return nc

        # --- one-time setup ---------------------------------------------
        ident = singles.tile([128, 128], f32)
        make_identity(nc, ident[:])
        ident_bf = singles.tile([128, 128], bf16)
        nc.vector.tensor_copy(ident_bf[:], ident[:])

        cw_sb = singles.tile([K, C], f32)
        nc.sync.dma_start(out=cw_sb[:], in_=cw_in[:])
        cent_sb = singles.tile([K, C], f32)
        nc.sync.dma_start(out=cent_sb[:], in_=cent_in[:])

        # conv_w^T [C, K] via PE transpose, then bf16; pack rhs1 = [cwT | I]
        rhs1 = singles.tile([C, K + 128], bf16)   # [128, 192]
        with tc.tile_pool(name="setup_ps", bufs=1, space="PSUM") as sps:
            cwt_ps = sps.tile([C, K], f32)
            nc.tensor.transpose(cwt_ps[:], cw_sb[:], ident[:K, :K])
            nc.scalar.copy(rhs1[:, :K], cwt_ps[:])
        nc.vector.tensor_copy(rhs1[:, K:], ident_bf[:])

        # indicator blocks for the ss matmuls: E[p, 8*n + m] = (m == n), bf16
        eind = singles.tile([C, NS * NS], bf16)
        nc.vector.memset(eind[:], 0.0)
        for n in range(NS):
            nc.vector.memset(eind[:, 9 * n : 9 * n + 1], 1.0)

        # --- per-core big buffers ---------------------------------------
        CH = 8                  # 512-col ss chunks per sample
        TF = SP // TP           # 32 tiles incl. pad tiles

        xbfp = ctx.enter_context(tc.tile_pool(name="xbf", bufs=3))
        xpool = ctx.enter_context(tc.tile_pool(name="x", bufs=2))
        x2pool = ctx.enter_context(tc.tile_pool(name="x2", bufs=2))
        stats = ctx.enter_context(tc.tile_pool(name="stats", bufs=2))
        ssps = ctx.enter_context(tc.tile_pool(name="ss_ps", bufs=2, space="PSUM"))
        trps = ctx.enter_context(tc.tile_pool(name="tr_ps", bufs=1, space="PSUM"))
        lgps = ctx.enter_context(tc.tile_pool(name="lg_ps", bufs=2, space="PSUM"))
        xtps = ctx.enter_context(tc.tile_pool(name="xt_ps", bufs=2, space="PSUM"))
        vps = ctx.enter_context(tc.tile_pool(name="v_ps", bufs=1, space="PSUM"))
        work = ctx.enter_context(tc.tile_pool(name="work", bufs=2))
        post = ctx.enter_context(tc.tile_pool(name="post", bufs=2))
        ln8 = post.tile([K, 1], f32, tag="ln8", bufs=1)
        nc.vector.memset(ln8[:], -2.0794415416798357)

        # quarter-granularity phase-2 ranges: finer ACT/DVE pipelining
        QRS = [(0, 8), (8, 15), (15, 22), (22, 29)]

        # persistent double/triple-buffered bf16 tiles with zeroed pads
        xbfs = [
            xbfp.tile([C, SP], bf16, tag=f"xbf{i}", bufs=1, name=f"xbf{i}")
            for i in range(4)
        ]
        x2s = [
            x2pool.tile([C, SP], bf16, tag=f"x2_{i}", bufs=1, name=f"x2_{i}")
            for i in range(2)
        ]
        for tl in xbfs:
            nc.gpsimd.memset(tl[:, S:], 0.0)
        for tl in x2s:
            nc.vector.memset(tl[:, 29 * TP :], 0.0)

        # sample 0 ramp: quarters in SEPARATE tiles so cast/square/ss start
        # as soon as the first 0.5MB lands (Tile deps are tile-granular)
        QW = 1024
        xq0 = [
            singles.tile([C, QW], f32, tag=f"xq0_{i}", name=f"xq0_{i}")
            for i in range(4)
        ]
        xbq0 = [
            singles.tile([C, QW], bf16, tag=f"xbq0_{i}", name=f"xbq0_{i}")
            for i in range(4)
        ]
        x2q0 = [
            singles.tile([C, QW], bf16, tag=f"x2q0_{i}", name=f"x2q0_{i}")
            for i in range(4)
        ]
        nc.gpsimd.memset(xbq0[3][:, S - 3 * QW :], 0.0)
        nc.vector.memset(x2q0[3][:, S - 3 * QW :], 0.0)

        def phase1(n):
            # load, bf16-cast, square, folded ss -> inv/norm per position
            if n == 0:
                for i in range(4):
                    w = min(QW, S - QW * i)
                    nc.sync.dma_start(
                        out=xq0[i][:, :w], in_=x_in[0, :, QW * i : QW * i + w]
                    )
                    nc.gpsimd.tensor_copy(
                        out=xbq0[i][:, :w], in_=xq0[i][:, :w]
                    )
                    nc.vector.tensor_mul(
                        x2q0[i][:, :w], xbq0[i][:, :w], xbq0[i][:, :w]
                    )
                xtiles = lambda t: xbq0[t // 8][
                    :, TP * (t % 8) : TP * (t % 8) + TP
                ]
                x2c = lambda j: x2q0[j // 2][:, 512 * (j % 2) : 512 * (j % 2 + 1)]
            else:
                xt = xpool.tile([C, S], f32, tag="xt")
                nc.sync.dma_start(out=xt[:], in_=x_in[n])
                xbf = xbfs[n % 4]
                if variant == "scalar_cast":
                    nc.scalar.copy(out=xbf[:, :S], in_=xt[:])
                else:
                    nc.gpsimd.tensor_copy(out=xbf[:, :S], in_=xt[:])
                x2 = x2s[n % 2]
                # bf16 square on DVE (2x mode), ScalarE kept for exp/drains
                nc.vector.tensor_mul(
                    x2[:, : 29 * TP], xbf[:, : 29 * TP], xbf[:, : 29 * TP]
                )
                xtiles = lambda t, xbf=xbf: xbf[:, TP * t : TP * t + TP]
                x2c = lambda j, x2=x2: x2[:, 512 * j : 512 * (j + 1)]
            # ss folded [chunk j -> row j]: one PSUM bank per sample
            ssb_ps = ssps.tile([NS, 512], f32, tag="ssb")
            for j in range(CH):
                nc.tensor.matmul(
                    ssb_ps[:],
                    eind[:, NS * j : NS * (j + 1)],
                    x2c(j),
                    start=(j == 0),
                    stop=(j == CH - 1),
                )
            ssb = stats.tile([NS, 512], f32, tag="ssb_sb")
            nc.scalar.copy(ssb[:], ssb_ps[:])
            # 4 transposes -> [128, (q,j)], then permuting copy -> t-order
            tr_ps = trps.tile([TP, 4 * NS], f32, tag="trp")
            for q in range(4):
                nc.tensor.transpose(
                    tr_ps[:, NS * q : NS * (q + 1)],
                    ssb[:, TP * q : TP * (q + 1)],
                    ident[:NS, :NS],
                )
            sst = stats.tile([TP, TF], f32, tag="sst")
            nc.scalar.copy(
                sst.rearrange("p (j q) -> p q j", q=4),
                tr_ps[:].rearrange("p (q j) -> p q j", j=NS),
            )
            # pad positions (s >= S): ss := 1 so ln/exp stay finite
            nc.gpsimd.affine_select(
                out=sst[:], in_=sst[:], pattern=[[-TP, TF]],
                compare_op=ALU.is_ge, fill=1.0, base=S - 1,
                channel_multiplier=-1,
            )
            invn = stats.tile([TP, TF], f32, tag="invn")
            normn = stats.tile([TP, TF], f32, tag="normn")
            # norm = exp(0.5*ln(ss)), inv = exp(-0.5*ln(ss)): same ACT
            # table set as softmax's Exp -- no table reloads anywhere
            nc.scalar.activation(sst[:], sst[:], AF.Ln)
            nc.scalar.activation(invn[:], sst[:], AF.Exp, scale=-0.5)
            nc.scalar.activation(normn[:], sst[:], AF.Exp, scale=0.5)
            # zero pad norms so their a'*norm asum contribution vanishes
            nc.gpsimd.affine_select(
                out=normn[:], in_=normn[:], pattern=[[-TP, TF]],
                compare_op=ALU.is_ge, fill=0.0, base=S - 1,
                channel_multiplier=-1,
            )
            return xtiles, invn, normn

        def phase2(n, xtiles, invn, normn):
            lsc = work.tile([TP, T, K], f32, tag="lsc")
            e = work.tile([TP, T, K], bf16, tag="e")
            ap = work.tile([TP, T, K], bf16, tag="ap")
            xts = work.tile([TP, T, 129], bf16, tag="xts")
            sume = work.tile([TP, T], f32, tag="sume")
            tsc = work.tile([TP, T], f32, tag="tsc")

            # norm column for mm2 rhs (pad-masked)
            nc.gpsimd.tensor_copy(out=xts[:, :, 128], in_=normn[:, :T])

            for half, (t0, t1) in enumerate(QRS):
                nt = t1 - t0
                lg = lgps.tile([TP, 8 * K], f32, tag="lg")
                for t in range(t0, t1):
                    nc.tensor.matmul(
                        lg[:, (t - t0) * K : (t - t0 + 1) * K],
                        xtiles(t),
                        rhs1[:, :K],
                        start=True,
                        stop=True,
                    )
                # X_T via PE transpose with bf16 PSUM out: drains get the
                # 16-bit 2x copy mode on ScalarE
                for g0 in range(t0, t1, 8):
                    g1 = min(g0 + 8, t1)
                    xtp = xtps.tile([TP, 8 * TP], bf16, tag="xtp")
                    for t in range(g0, g1):
                        nc.tensor.transpose(
                            xtp[:, (t - g0) * TP : (t - g0 + 1) * TP],
                            xtiles(t),
                            ident_bf[:],
                        )
                    nc.scalar.copy(
                        xts[:, g0:g1, :TP],
                        xtp[:, : (g1 - g0) * TP].rearrange(
                            "p (t c) -> p t c", c=TP
                        ),
                    )
                # lsc = logits * inv (step-0 broadcast along K)
                inv_b = invn[:, t0:t1].rearrange(
                    "p (t o) -> p t o", o=1
                ).broadcast_to([TP, nt, K])
                nc.vector.tensor_tensor(
                    out=lsc[:, t0:t1, :],
                    in0=lg[:, : nt * K].rearrange("p (t k) -> p t k", k=K),
                    in1=inv_b,
                    op=ALU.mult,
                )

            # half-granularity softmax chain: shortens the per-sample
            # latency (exp/sume/a'/mm2 of half A run while half B computes)
            tscb = work.tile([TP, T], bf16, tag="tscb")
            vp = vps.tile([K, 130], f32, tag="vp")
            for t0, t1 in QRS:
                nc.scalar.activation(e[:, t0:t1, :], lsc[:, t0:t1, :], AF.Exp)
                nc.vector.tensor_reduce(
                    sume[:, t0:t1], e[:, t0:t1, :], axis=AX.X, op=ALU.add
                )
                nc.vector.reciprocal(tsc[:, t0:t1], sume[:, t0:t1])
                nc.vector.tensor_tensor(
                    out=tsc[:, t0:t1], in0=tsc[:, t0:t1], in1=invn[:, t0:t1],
                    op=ALU.mult,
                )
                nc.vector.tensor_copy(tscb[:, t0:t1], tsc[:, t0:t1])
                nc.gpsimd.tensor_tensor(
                    out=ap[:, t0:t1, :],
                    in0=e[:, t0:t1, :],
                    in1=tscb[:, t0:t1].rearrange(
                        "p (t o) -> p t o", o=1
                    ).broadcast_to([TP, t1 - t0, K]),
                    op=ALU.mult,
                )
                for t in range(t0, t1):
                    nc.tensor.matmul(
                        vp[:, :129],
                        ap[:, t, :],
                        xts[:, t, :],
                        start=(t == 0),
                        stop=(t == T - 1),
                    )
            wv = post.tile([K, 129], f32, tag="wv")
            nc.scalar.copy(wv[:], vp[:, :129])

            # per-sample post: vlad = vlad1 - asum*cent, intra-norm, /8
            vz = post.tile([K, C], f32, tag="vz")
            nc.vector.tensor_scalar_mul(vz[:], cent_sb[:], wv[:, 128:129])
            nc.gpsimd.tensor_tensor(
                out=wv[:, :C], in0=wv[:, :C], in1=vz[:], op=ALU.subtract
            )
            rr = post.tile([K, 1], f32, tag="rr")
            nc.vector.tensor_mul(vz[:], wv[:, :C], wv[:, :C])
            nc.vector.tensor_reduce(rr[:], vz[:], axis=AX.X, op=ALU.add)
            # 0.125/sqrt(r) = exp(-0.5*ln(r) + ln(0.125)): same ACT table set
            nc.scalar.activation(rr[:], rr[:], AF.Ln)
            nc.scalar.activation(rr[:], rr[:], AF.Exp, scale=-0.5, bias=ln8[:])
            ov = post.tile([K, C], f32, tag="ov")
            nc.vector.tensor_scalar_mul(ov[:], wv[:, :C], rr[:])
            nc.sync.dma_start(
                out=out_ext[n].rearrange("(k c) -> k c", c=C), in_=ov[:]
            )

        # software-pipelined: emit phase1(n+1) before phase2(n) so no
        # engine's program order makes sample n+1's prep wait on sample n
        for _rep in range(reps):
            carry = phase1(0)
            for n in range(NS):
                nxt = phase1(n + 1) if n + 1 < NS else None
                if variant != "phase1":
                    phase2(n, *carry)
                carry = nxt

        if variant == "phase1":
            ov0 = singles.tile([K, C], f32)
            nc.vector.memset(ov0[:], 0.0)
            for n in range(NS):
                nc.sync.dma_start(
                    out=out_ext[n].rearrange("(k c) -> k c", c=C), in_=ov0[:]
                )
            return nc

    return nc


def kernel(x, conv_w, centroids):
    from concourse.bass_utils import run_bass_kernel_spmd

    if "nc" not in _cache:
        nc = build_nc()
        nc.finalize()
        _cache["nc"] = nc
    nc = _cache["nc"]

    xr = np.ascontiguousarray(x.reshape(N, C, S), dtype=np.float32)
    cw = np.ascontiguousarray(conv_w, dtype=np.float32)
    ct = np.ascontiguousarray(centroids, dtype=np.float32)
    in_maps = [
        {"x": xr[i * NS : (i + 1) * NS], "conv_w": cw, "centroids": ct}
        for i in range(NCORES)
    ]
    res = run_bass_kernel_spmd(nc, in_maps, list(range(NCORES)))
    outs = [res.results[i]["out"] for i in range(NCORES)]
    return np.concatenate(outs, axis=0).reshape(N, K * C)


# revision 76
# speedup vs baseline: 1.0095x; 1.0095x over previous
"""NetVLAD Trainium2 kernel (Bass/Tile), data-parallel over batch on 8 cores.

Problem (hardcoded): x [64, 128, 60, 60] f32, conv_w [64, 128] f32,
centroids [64, 128] f32 -> out [64, 8192] f32.

Per core: 8 samples, software-pipelined (phase1 of sample n+1 is emitted
before phase2 of sample n so no engine's program order stalls the next
sample). S = 3600 spatial positions padded to SP = 4096 (8 x 512 ss
chunks); phase 2 only touches the 29 real 128-position tiles.

Per sample (layouts chosen so every reduction is a free-dim reduction and
ScalarE stays on the natural_log_exp table set -- no ACT table reloads):
  phase1: ss[s] = sum_c x^2 via indicator-matmuls folded into one PSUM
          bank [chunk, 512]; 4 PE transposes + permuting copy -> t-order;
          inv = exp(-0.5 ln ss), norm = exp(0.5 ln ss), pads masked by
          affine_select (ss:=1, norm:=0)
  phase2: mm1 lhsT=x_tile[bf16] -> logits [s,64]; PE transposes -> X_T
          (bf16 PSUM, 2x-mode drains); lsc = logits*inv (DVE bcast);
          e = exp(lsc); sume = sum_k e; a' = e*(inv/sume) (GPSIMD);
          mm2 lhsT=a', rhs=[X_T | norm] -> [64,129] = [vlad1^T | asum];
          v = vlad1 - asum*cent; out = v * (0.125/sqrt(sum v^2)) via
          exp/ln (global L2 norm of 64 unit rows == 8 exactly)
"""

import os
import numpy as np

N, C, H, W, K = 64, 128, 60, 60, 64
S = H * W            # 3600
NS = 8               # samples per core
NCORES = 8
TP = 128             # s-tile size
T = 29               # real (non-pad) tiles per sample
SP = 4096            # padded positions: 8 x 512 ss-chunks, 32 tiles
PAD = SP - S         # 496

_cache = {}


def build_nc(variant="full", reps=1):
    # variant: full | dma_only | phase1 | no_mm2 | scalar_cast
    # reps>1 repeats the whole pipeline (for on-device timing via slope)
    import concourse.bass as bass
    import concourse.bacc as bacc
    import concourse.tile as tile
    from concourse import mybir
    from concourse.masks import make_identity

    f32 = mybir.dt.float32
    bf16 = mybir.dt.bfloat16
    AF = mybir.ActivationFunctionType
    ALU = mybir.AluOpType
    AX = mybir.AxisListType

    nc = bacc.Bacc("TRN2")
    x_in = nc.declare_dram_parameter("x", [NS, C, S], f32, isOutput=False)
    cw_in = nc.declare_dram_parameter("conv_w", [K, C], f32, isOutput=False)
    cent_in = nc.declare_dram_parameter("centroids", [K, C], f32, isOutput=False)
    out_ext = nc.declare_dram_parameter("out", [NS, K * C], f32, isOutput=True)

    from contextlib import ExitStack

    with tile.TileContext(nc) as tc, ExitStack() as ctx:
        singles = ctx.enter_context(tc.tile_pool(name="singles", bufs=1))

        if variant == "dma_only":
            acc = singles.tile([C, SP], f32)
            for n in range(NS):
                xt = singles.tile([C, SP], f32, tag=f"x{n}")
                nc.sync.dma_start(out=xt[:, :S], in_=x_in[n])
            nc.vector.tensor_copy(acc[:], xt[:])
            ov0 = singles.tile([K, C], f32)
            nc.vector.tensor_copy(ov0[:], acc[:K, :C])
            for n in range(NS):
                nc.sync.dma_start(
                    out=out_ext[n].rearrange("(k c) -> k c", c=C), in_=ov0[:]
                )
            return nc

        # --- one-time setup ---------------------------------------------
        ident = singles.tile([128, 128], f32)
        make_identity(nc, ident[:])
        ident_bf = singles.tile([128, 128], bf16)
        nc.vector.tensor_copy(ident_bf[:], ident[:])

        cw_sb = singles.tile([K, C], f32)
        nc.sync.dma_start(out=cw_sb[:], in_=cw_in[:])
        cent_sb = singles.tile([K, C], f32)
        nc.sync.dma_start(out=cent_sb[:], in_=cent_in[:])

        # conv_w^T [C, K] via PE transpose, then bf16; pack rhs1 = [cwT | I]
        rhs1 = singles.tile([C, K + 128], bf16)   # [128, 192]
        with tc.tile_pool(name="setup_ps", bufs=1, space="PSUM") as sps:
            cwt_ps = sps.tile([C, K], f32)
            nc.tensor.transpose(cwt_ps[:], cw_sb[:], ident[:K, :K])
            nc.scalar.copy(rhs1[:, :K], cwt_ps[:])
        nc.vector.tensor_copy(rhs1[:, K:], ident_bf[:])

        # indicator blocks for the ss matmuls: E[p, 8*n + m] = (m == n), bf16
        eind = singles.tile([C, NS * NS], bf16)
        nc.vector.memset(eind[:], 0.0)
        for n in range(NS):
            nc.vector.memset(eind[:, 9 * n : 9 * n + 1], 1.0)

        # --- per-core big buffers ---------------------------------------
        CH = 8                  # 512-col ss chunks per sample
        TF = SP // TP           # 32 tiles incl. pad tiles

        xbfp = ctx.enter_context(tc.tile_pool(name="xbf", bufs=3))
        xpool = ctx.enter_context(tc.tile_pool(name="x", bufs=2))
        x2pool = ctx.enter_context(tc.tile_pool(name="x2", bufs=2))
        stats = ctx.enter_context(tc.tile_pool(name="stats", bufs=2))
        ssps = ctx.enter_context(tc.tile_pool(name="ss_ps", bufs=2, space="PSUM"))
        trps = ctx.enter_context(tc.tile_pool(name="tr_ps", bufs=1, space="PSUM"))
        lgps = ctx.enter_context(tc.tile_pool(name="lg_ps", bufs=2, space="PSUM"))
        xtps = ctx.enter_context(tc.tile_pool(name="xt_ps", bufs=2, space="PSUM"))
        vps = ctx.enter_context(tc.tile_pool(name="v_ps", bufs=1, space="PSUM"))
        work = ctx.enter_context(tc.tile_pool(name="work", bufs=2))
        post = ctx.enter_context(tc.tile_pool(name="post", bufs=2))
        ln8 = post.tile([K, 1], f32, tag="ln8", bufs=1)
        nc.vector.memset(ln8[:], -2.0794415416798357)

        # quarter-granularity phase-2 ranges: finer ACT/DVE pipelining
        QRS = [(0, 8), (8, 15), (15, 22), (22, 29)]

        # persistent double/triple-buffered bf16 tiles with zeroed pads
        xbfs = [
            xbfp.tile([C, SP], bf16, tag=f"xbf{i}", bufs=1, name=f"xbf{i}")
            for i in range(4)
        ]
        x2s = [
            x2pool.tile([C, SP], bf16, tag=f"x2_{i}", bufs=1, name=f"x2_{i}")
            for i in range(2)
        ]
        for tl in xbfs:
            nc.gpsimd.memset(tl[:, S:], 0.0)
        for tl in x2s:
            nc.vector.memset(tl[:, 29 * TP :], 0.0)

        # sample 0 ramp: quarters in SEPARATE tiles so cast/square/ss start
        # as soon as the first 0.5MB lands (Tile deps are tile-granular)
        QW = 1024
        xq0 = [
            singles.tile([C, QW], f32, tag=f"xq0_{i}", name=f"xq0_{i}")
            for i in range(4)
        ]
        xbq0 = [
            singles.tile([C, QW], bf16, tag=f"xbq0_{i}", name=f"xbq0_{i}")
            for i in range(4)
        ]
        x2q0 = [
            singles.tile([C, QW], bf16, tag=f"x2q0_{i}", name=f"x2q0_{i}")
            for i in range(4)
        ]
        nc.gpsimd.memset(xbq0[3][:, S - 3 * QW :], 0.0)
        nc.vector.memset(x2q0[3][:, S - 3 * QW :], 0.0)

        def phase1(n):
            # load, bf16-cast, square, folded ss -> inv/norm per position
            if n == 0:
                for i in range(4):
                    w = min(QW, S - QW * i)
                    nc.sync.dma_start(
                        out=xq0[i][:, :w], in_=x_in[0, :, QW * i : QW * i + w]
                    )
                    nc.gpsimd.tensor_copy(
                        out=xbq0[i][:, :w], in_=xq0[i][:, :w]
                    )
                    nc.vector.tensor_mul(
                        x2q0[i][:, :w], xbq0[i][:, :w], xbq0[i][:, :w]
                    )
                xtiles = lambda t: xbq0[t // 8][
                    :, TP * (t % 8) : TP * (t % 8) + TP
                ]
                x2c = lambda j: x2q0[j // 2][:, 512 * (j % 2) : 512 * (j % 2 + 1)]
            else:
                xt = xpool.tile([C, S], f32, tag="xt")
                nc.sync.dma_start(out=xt[:], in_=x_in[n])
                xbf = xbfs[n % 4]
                if variant == "scalar_cast":
                    nc.scalar.copy(out=xbf[:, :S], in_=xt[:])
                else:
                    nc.gpsimd.tensor_copy(out=xbf[:, :S], in_=xt[:])
                x2 = x2s[n % 2]
                # bf16 square on DVE (2x mode), ScalarE kept for exp/drains
                nc.vector.tensor_mul(
                    x2[:, : 29 * TP], xbf[:, : 29 * TP], xbf[:, : 29 * TP]
                )
                xtiles = lambda t, xbf=xbf: xbf[:, TP * t : TP * t + TP]
                x2c = lambda j, x2=x2: x2[:, 512 * j : 512 * (j + 1)]
            # ss folded [chunk j -> row j]: one PSUM bank per sample
            ssb_ps = ssps.tile([NS, 512], f32, tag="ssb")
            for j in range(CH):
                nc.tensor.matmul(
                    ssb_ps[:],
                    eind[:, NS * j : NS * (j + 1)],
                    x2c(j),
                    start=(j == 0),
                    stop=(j == CH - 1),
                )
            ssb = stats.tile([NS, 512], f32, tag="ssb_sb")
            nc.scalar.copy(ssb[:], ssb_ps[:])
            # 4 transposes -> [128, (q,j)], then permuting copy -> t-order
            tr_ps = trps.tile([TP, 4 * NS], f32, tag="trp")
            for q in range(4):
                nc.tensor.transpose(
                    tr_ps[:, NS * q : NS * (q + 1)],
                    ssb[:, TP * q : TP * (q + 1)],
                    ident[:NS, :NS],
                )
            sst = stats.tile([TP, TF], f32, tag="sst")
            nc.scalar.copy(
                sst.rearrange("p (j q) -> p q j", q=4),
                tr_ps[:].rearrange("p (q j) -> p q j", j=NS),
            )
            # pad positions (s >= S): ss := 1 so ln/exp stay finite
            nc.gpsimd.affine_select(
                out=sst[:], in_=sst[:], pattern=[[-TP, TF]],
                compare_op=ALU.is_ge, fill=1.0, base=S - 1,
                channel_multiplier=-1,
            )
            invn = stats.tile([TP, TF], f32, tag="invn")
            normn = stats.tile([TP, TF], f32, tag="normn")
            # norm = exp(0.5*ln(ss)), inv = exp(-0.5*ln(ss)): same ACT
            # table set as softmax's Exp -- no table reloads anywhere
            nc.scalar.activation(sst[:], sst[:], AF.Ln)
            nc.scalar.activation(invn[:], sst[:], AF.Exp, scale=-0.5)
            nc.scalar.activation(normn[:], sst[:], AF.Exp, scale=0.5)
            # zero pad norms so their a'*norm asum contribution vanishes
            nc.gpsimd.affine_select(
                out=normn[:], in_=normn[:], pattern=[[-TP, TF]],
                compare_op=ALU.is_ge, fill=0.0, base=S - 1,
                channel_multiplier=-1,
            )
            return xtiles, invn, normn

        def phase2(n, xtiles, invn, normn):
            lsc = work.tile([TP, T, K], f32, tag="lsc")
            e = work.tile([TP, T, K], bf16, tag="e")
            ap = work.tile([TP, T, K], bf16, tag="ap")
            xts = work.tile([TP, T, 129], bf16, tag="xts")
            sume = work.tile([TP, T], f32, tag="sume")
            tsc = work.tile([TP, T], f32, tag="tsc")

            # norm column for mm2 rhs (pad-masked)
            nc.gpsimd.tensor_copy(out=xts[:, :, 128], in_=normn[:, :T])

            for half, (t0, t1) in enumerate(QRS):
                nt = t1 - t0
                lg = lgps.tile([TP, 8 * K], f32, tag="lg")
                for t in range(t0, t1):
                    nc.tensor.matmul(
                        lg[:, (t - t0) * K : (t - t0 + 1) * K],
                        xtiles(t),
                        rhs1[:, :K],
                        start=True,
                        stop=True,
                    )
                # X_T via PE transpose with bf16 PSUM out: drains get the
                # 16-bit 2x copy mode on ScalarE
                for g0 in range(t0, t1, 8):
                    g1 = min(g0 + 8, t1)
                    xtp = xtps.tile([TP, 8 * TP], bf16, tag="xtp")
                    for t in range(g0, g1):
                        nc.tensor.transpose(
                            xtp[:, (t - g0) * TP : (t - g0 + 1) * TP],
                            xtiles(t),
                            ident_bf[:],
                        )
                    nc.scalar.copy(
                        xts[:, g0:g1, :TP],
                        xtp[:, : (g1 - g0) * TP].rearrange(
                            "p (t c) -> p t c", c=TP
                        ),
                    )
                # lsc = logits * inv (step-0 broadcast along K)
                inv_b = invn[:, t0:t1].rearrange(
                    "p (t o) -> p t o", o=1
                ).broadcast_to([TP, nt, K])
                nc.vector.tensor_tensor(
                    out=lsc[:, t0:t1, :],
                    in0=lg[:, : nt * K].rearrange("p (t k) -> p t k", k=K),
                    in1=inv_b,
                    op=ALU.mult,
                )

            # half-granularity softmax chain: shortens the per-sample
            # latency (exp/sume/a'/mm2 of half A run while half B computes)
            tscb = work.tile([TP, T], bf16, tag="tscb")
            vp = vps.tile([K, 130], f32, tag="vp")
            # exp per half (lower fixed overhead on the pacing ScalarE);
            # the DVE/GPSIMD chain below stays at quarter granularity
            nc.scalar.activation(e[:, :15, :], lsc[:, :15, :], AF.Exp)
            nc.scalar.activation(e[:, 15:, :], lsc[:, 15:, :], AF.Exp)
            for t0, t1 in QRS:
                nc.vector.tensor_reduce(
                    sume[:, t0:t1], e[:, t0:t1, :], axis=AX.X, op=ALU.add
                )
                nc.vector.reciprocal(tsc[:, t0:t1], sume[:, t0:t1])
                nc.vector.tensor_tensor(
                    out=tsc[:, t0:t1], in0=tsc[:, t0:t1], in1=invn[:, t0:t1],
                    op=ALU.mult,
                )
                nc.vector.tensor_copy(tscb[:, t0:t1], tsc[:, t0:t1])
                nc.gpsimd.tensor_tensor(
                    out=ap[:, t0:t1, :],
                    in0=e[:, t0:t1, :],
                    in1=tscb[:, t0:t1].rearrange(
                        "p (t o) -> p t o", o=1
                    ).broadcast_to([TP, t1 - t0, K]),
                    op=ALU.mult,
                )
                for t in range(t0, t1):
                    nc.tensor.matmul(
                        vp[:, :129],
                        ap[:, t, :],
                        xts[:, t, :],
                        start=(t == 0),
                        stop=(t == T - 1),
                    )
            wv = post.tile([K, 129], f32, tag="wv")
            nc.scalar.copy(wv[:], vp[:, :129])

            # per-sample post: vlad = vlad1 - asum*cent, intra-norm, /8
            vz = post.tile([K, C], f32, tag="vz")
            nc.vector.tensor_scalar_mul(vz[:], cent_sb[:], wv[:, 128:129])
            nc.gpsimd.tensor_tensor(
                out=wv[:, :C], in0=wv[:, :C], in1=vz[:], op=ALU.subtract
            )
            rr = post.tile([K, 1], f32, tag="rr")
            nc.vector.tensor_mul(vz[:], wv[:, :C], wv[:, :C])
            nc.vector.tensor_reduce(rr[:], vz[:], axis=AX.X, op=ALU.add)
            # 0.125/sqrt(r) = exp(-0.5*ln(r) + ln(0.125)): same ACT table set
            nc.scalar.activation(rr[:], rr[:], AF.Ln)
            nc.scalar.activation(rr[:], rr[:], AF.Exp, scale=-0.5, bias=ln8[:])
            ov = post.tile([K, C], f32, tag="ov")
            nc.vector.tensor_scalar_mul(ov[:], wv[:, :C], rr[:])
            nc.sync.dma_start(
                out=out_ext[n].rearrange("(k c) -> k c", c=C), in_=ov[:]
            )

        # software-pipelined: emit phase1(n+1) before phase2(n) so no
        # engine's program order makes sample n+1's prep wait on sample n
        for _rep in range(reps):
            carry = phase1(0)
            for n in range(NS):
                nxt = phase1(n + 1) if n + 1 < NS else None
                if variant != "phase1":
                    phase2(n, *carry)
                carry = nxt

        if variant == "phase1":
            ov0 = singles.tile([K, C], f32)
            nc.vector.memset(ov0[:], 0.0)
            for n in range(NS):
                nc.sync.dma_start(
                    out=out_ext[n].rearrange("(k c) -> k c", c=C), in_=ov0[:]
                )
            return nc

    return nc


def kernel(x, conv_w, centroids):
    from concourse.bass_utils import run_bass_kernel_spmd

    if "nc" not in _cache:
        nc = build_nc()
        nc.finalize()
        _cache["nc"] = nc
    nc = _cache["nc"]

    xr = np.ascontiguousarray(x.reshape(N, C, S), dtype=np.float32)
    cw = np.ascontiguousarray(conv_w, dtype=np.float32)
    ct = np.ascontiguousarray(centroids, dtype=np.float32)
    in_maps = [
        {"x": xr[i * NS : (i + 1) * NS], "conv_w": cw, "centroids": ct}
        for i in range(NCORES)
    ]
    res = run_bass_kernel_spmd(nc, in_maps, list(range(NCORES)))
    outs = [res.results[i]["out"] for i in range(NCORES)]
    return np.concatenate(outs, axis=0).reshape(N, K * C)
